# revision 1
# baseline (speedup 1.0000x reference)
"""Biaffine edge attention on 8 Trainium2 NeuronCores.

out[b,i,j] = head[b,i,:] @ edge_U @ dep[b,j,:] + head[b,i,:]@w1 + dep[b,j,:]@w2 + b0

Sharding: data-parallel over batch (B=8, one batch per core). Per core:
  HT = transpose(head[b])                (PE identity-transpose, fp32r)
  T1T[k,i] = sum_d U[d,k] * HT[d,i]      (fp32r matmul, lhsT=U natural layout)
  PT = transpose(dep[b])
  out[i,j] = sum_k T1T[k,i] * PT[k,j] + s_head[i] + s_dep[j] + b0

Matmuls/transposes run in float32r (full PE rate at free dim >= 512, ~fp32
precision). DMA loads go straight into fp32r tiles (verified numerically OK
on HW). Transposes of the second half of H / of P are interleaved into the
matmul instruction stream so they execute at the warm (2.4 GHz) PE clock --
isolated transpose-mode work does not trip the HAM un-throttle.
"""

import numpy as np

import concourse.bass as bass
import concourse.mybir as mybir
import concourse.tile as tile
from concourse import bacc
from concourse.bass_utils import run_bass_kernel_spmd
from concourse.masks import make_identity

B, S, D = 8, 1024, 1024
P = 128
SO = S // P  # 8
DO = D // P  # 8
NH = 512     # matmul free-dim tile (one fp32 PSUM bank)
F32 = mybir.dt.float32
F32R = mybir.dt.float32r
ADD = mybir.AluOpType.add
MULT = mybir.AluOpType.mult

_CACHE = {}


def build_nc(variant=4):
    nc = bacc.Bacc(None, target_bir_lowering=False)

    head = nc.dram_tensor("head", [S, D], F32R, kind="ExternalInput")
    dep = nc.dram_tensor("dep", [S, D], F32R, kind="ExternalInput")
    # host-relayouted U: u_prep[kt, dd, do, k] = U[do*P+dd, kt*P+k] so each
    # kt column-block is one contiguous 4KB chunk per partition
    edge_u = nc.dram_tensor("edge_u", [DO, P, DO, P], F32R, kind="ExternalInput")
    w_head_bc = nc.dram_tensor("w_head_bc", [P, D], F32, kind="ExternalInput")
    w_dep_col = nc.dram_tensor("w_dep_col", [P, DO], F32R, kind="ExternalInput")
    bias0 = nc.dram_tensor("bias0", [1, 1], F32, kind="ExternalInput")
    out = nc.dram_tensor("out", [S, S], F32, kind="ExternalOutput")

    with tile.TileContext(nc) as tc:
        with (
            tc.tile_pool(name="const", bufs=1) as const,
            tc.tile_pool(name="big", bufs=1) as big,
            tc.tile_pool(name="stage", bufs=8) as stage,
            tc.tile_pool(name="scratch", bufs=2) as scratch,
            tc.tile_pool(name="outp", bufs=4) as outp,
            tc.tile_pool(name="tp_ps", bufs=2, space="PSUM") as tp_ps,
            tc.tile_pool(name="mm_ps", bufs=5, space="PSUM") as mm_ps,
            tc.tile_pool(name="sm_ps", bufs=1, space="PSUM") as sm_ps,
        ):
            ident_raw = const.tile([P, P], F32)
            make_identity(nc, ident_raw)
            ident = const.tile([P, P], F32R)
            nc.vector.tensor_copy(ident[:], ident_raw[:])
            b_raw = const.tile([1, 1], F32)
            wd_sb = const.tile([P, DO], F32R)
            wh_sb = const.tile([P, D], F32)
            shead_col = const.tile([P, SO], F32)
            sdep_row = const.tile([1, S], F32)
            sdep_full = const.tile([P, S], F32)

            u_sb = big.tile([P, DO, D], F32R, tag="u")      # [dd, do, k]
            ht_sb = big.tile([P, DO, S], F32R, tag="ht")    # [dd, do, i]
            pt_sb = big.tile([P, DO, S], F32R, tag="pt")    # [kk, kt, j]
            t1t_sb = big.tile([P, DO, S], F32R, tag="t1t")  # [kk, kt, i]

            # ---------- DMA emission (sync ring is FIFO: order = priority) --
            h_stage = [None] * SO
            p_stage = [None] * SO

            def load_stage(src, arr, idx, split=1):
                t = stage.tile([P, D], F32R, tag="stage")
                w = D // split
                for s in range(split):
                    nc.sync.dma_start(
                        t[:, s * w:(s + 1) * w],
                        src[idx * P:(idx + 1) * P, s * w:(s + 1) * w],
                    )
                arr[idx] = t

            # All loads on the sync HWDGE ring (FIFO dispatch). U column-block
            # loads have expensive descriptor generation (~2-5 us dispatch), so
            # interleave them with the H stages to rate-match consumption:
            # phase A eats h0..h3, phase B eats one u column + one h stage per
            # kt group.
            def load_u_col(kt):
                nc.sync.dma_start(
                    u_sb[:, :, kt * P:(kt + 1) * P], edge_u[kt]
                )

            load_stage(head, h_stage, 0, split=2)
            for io in range(1, 4):
                load_stage(head, h_stage, io)
            load_u_col(0)
            load_u_col(1)
            load_u_col(2)
            for io in range(4, SO):
                load_stage(head, h_stage, io)
                load_u_col(io - 1)
            load_u_col(7)
            nc.sync.dma_start(wh_sb[:], w_head_bc[:])
            nc.sync.dma_start(wd_sb[:], w_dep_col[:])
            nc.sync.dma_start(b_raw[:], bias0[:])

            # ---------- helpers ----------
            copy_eng = [0]

            def copy(dst, src, eng=None):
                if eng is None:
                    eng = "act" if copy_eng[0] % 2 == 0 else "dve"
                    copy_eng[0] += 1
                if eng == "act":
                    nc.scalar.copy(dst, src)
                else:
                    nc.vector.tensor_copy(dst, src)

            def tpose_group(stages, idx, q4, dst_big, eng=None):
                """Transpose 4 [P,P] blocks (dims q4*4..q4*4+3) of stage idx."""
                ps = tp_ps.tile([P, NH], F32R, tag="tp")
                for q in range(4):
                    do = q4 * 4 + q
                    nc.tensor.transpose(
                        ps[:, q * P:(q + 1) * P],
                        stages[idx][:, do * P:(do + 1) * P],
                        ident[:],
                    )
                dst = dst_big[:, q4 * 4:q4 * 4 + 4, idx * P:(idx + 1) * P]
                copy(dst, ps[:].rearrange("p (q c) -> p q c", q=4), eng)

            def mm1_group(kt, ih, eng=None):
                ps = mm_ps.tile([P, NH], F32, tag="mm")
                for do in range(DO):
                    nc.tensor.matmul(
                        ps[:],
                        u_sb[:, do, kt * P:(kt + 1) * P],
                        ht_sb[:, do, ih * NH:(ih + 1) * NH],
                        start=(do == 0),
                        stop=(do == DO - 1),
                    )
                copy(t1t_sb[:, kt, ih * NH:(ih + 1) * NH], ps[:], eng)

            def shead_ops(io):
                sc = scratch.tile([P, D], F32, tag="scratch")
                nc.vector.tensor_mul(sc[:], h_stage[io][:].bitcast(F32), wh_sb[:])
                nc.vector.reduce_sum(
                    shead_col[:, io:io + 1], sc[:], axis=mybir.AxisListType.X
                )

            def sdep_ops(jh):
                ps = sm_ps.tile([P, NH], F32, tag="sm")
                for kt in range(DO):
                    nc.tensor.matmul(
                        ps[0:1, :],
                        wd_sb[:, kt:kt + 1],
                        pt_sb[:, kt, jh * NH:(jh + 1) * NH],
                        start=(kt == 0),
                        stop=(kt == DO - 1),
                    )
                nc.vector.tensor_scalar(
                    sdep_row[0:1, jh * NH:(jh + 1) * NH],
                    ps[0:1, :], b_raw[0:1, 0:1], None, ADD,
                )
                nc.gpsimd.partition_broadcast(
                    sdep_full[:, jh * NH:(jh + 1) * NH],
                    sdep_row[0:1, jh * NH:(jh + 1) * NH],
                )

            def mm2_group(it, jh, split=1):
                ps = mm_ps.tile([P, NH], F32, tag="mm")
                for kt in range(DO):
                    nc.tensor.matmul(
                        ps[:],
                        t1t_sb[:, kt, it * P:(it + 1) * P],
                        pt_sb[:, kt, jh * NH:(jh + 1) * NH],
                        start=(kt == 0),
                        stop=(kt == DO - 1),
                    )
                ot = outp.tile([P, NH], F32, tag="out")
                w = NH // split
                for s in range(split):
                    sl = slice(s * w, (s + 1) * w)
                    nc.vector.scalar_tensor_tensor(
                        out=ot[:, sl], in0=ps[:, sl],
                        scalar=shead_col[:, it:it + 1],
                        in1=sdep_full[:, jh * NH + s * w:jh * NH + (s + 1) * w],
                        op0=ADD, op1=ADD,
                    )
                    nc.sync.dma_start(
                        out[it * P:(it + 1) * P,
                            jh * NH + s * w:jh * NH + (s + 1) * w],
                        ot[:, sl],
                    )

            # ---------- phase A: transpose H rows io 0..3 ----------
            for io in range(4):
                for q4 in range(2):
                    tpose_group(h_stage, io, q4, ht_sb)

            # ---------- phase B: mm1 ih=0, interleave H transposes io 4..7 --
            pend = [(io, q4) for io in range(4, SO) for q4 in range(2)]
            for kt in range(DO):
                if kt >= DO - 2:
                    io, q4 = pend.pop(0)
                    tpose_group(h_stage, io, q4, ht_sb)
                mm1_group(kt, 0)
                if kt < DO - 2:
                    io, q4 = pend.pop(0)
                    tpose_group(h_stage, io, q4, ht_sb)

            # s_head on DVE (after phase-B copies in DVE program order, so the
            # early transpose-copy drain is not blocked behind the wh_sb DMA)
            for io in range(SO):
                shead_ops(io)

            # ---------- P loads (reuse stage slots as they free up) ----------
            for jo in range(SO):
                load_stage(dep, p_stage, jo)

            # ---------- phase C: mm1 ih=1, interleave P transposes jo 0..3 --
            # all copies on ACT: DVE is busy with the s_head mult/reduce block
            pend = [(jo, q4) for jo in range(4) for q4 in range(2)]
            for kt in range(DO):
                mm1_group(kt, 1, eng="act")
                jo, q4 = pend.pop(0)
                tpose_group(p_stage, jo, q4, pt_sb, eng="act")

            # ---------- phase D/E: sdep half 0, mm2 jh=0 + P transposes 4..7
            sdep_ops(0)
            pend = [(jo, q4) for jo in range(4, SO) for q4 in range(2)]
            for it in range(SO):
                mm2_group(it, 0)
                jo, q4 = pend.pop(0)
                tpose_group(p_stage, jo, q4, pt_sb, eng="act")

            # ---------- phase F/G: sdep half 1, mm2 jh=1 ----------
            sdep_ops(1)
            for it in range(SO):
                # split the last group's epilogue so the tail latency chain
                # (STT -> out DMA) is half as long
                mm2_group(it, 1, split=(4 if it == SO - 1 else 1))

    nc.compile()
    return nc


def _get_nc(variant=4):
    key = ("nc", variant)
    if key not in _CACHE:
        _CACHE[key] = build_nc(variant)
    return _CACHE[key]


def _in_maps(head, dep, edge_U, edge_W, edge_b):
    # pull everything to host numpy first (inputs may be jax device arrays)
    head = np.asarray(head, dtype=np.float32)
    dep = np.asarray(dep, dtype=np.float32)
    edge_U = np.asarray(edge_U, dtype=np.float32)
    w = np.asarray(edge_W, dtype=np.float32).reshape(-1)
    w1, w2 = w[:D], w[D:]
    w_head_bc = np.ascontiguousarray(np.broadcast_to(w1[None, :], (P, D)))
    w_dep_col = np.ascontiguousarray(w2.reshape(DO, P).T)  # [kk, kt]
    b0 = np.asarray(edge_b, dtype=np.float32).reshape(1, 1)
    u_prep = np.ascontiguousarray(
        np.asarray(edge_U, dtype=np.float32)
        .reshape(DO, P, DO, P).transpose(2, 1, 0, 3)
    )
    maps = []
    for b in range(B):
        maps.append({
            "head": np.ascontiguousarray(head[b], dtype=np.float32),
            "dep": np.ascontiguousarray(dep[b], dtype=np.float32),
            "edge_u": u_prep,
            "w_head_bc": w_head_bc,
            "w_dep_col": w_dep_col,
            "bias0": b0,
        })
    return maps


def kernel(head, dep, edge_U, edge_W, edge_b, **run_kwargs):
    nc = _get_nc()
    maps = _in_maps(head, dep, edge_U, edge_W, edge_b)
    res = run_bass_kernel_spmd(nc, maps, core_ids=list(range(B)), **run_kwargs)
    out = np.stack([res.results[c]["out"] for c in range(B)], axis=0)
    if run_kwargs:
        _CACHE["last_result"] = res
    return out



# revision 4
# speedup vs baseline: 1.1200x; 1.1200x over previous
"""Biaffine edge attention on 8 Trainium2 NeuronCores.

out[b,i,j] = head[b,i,:] @ edge_U @ dep[b,j,:] + head[b,i,:]@w1 + dep[b,j,:]@w2 + b0

Sharding: data-parallel over batch (B=8, one batch per core).

v2 design: everything in bf16 (host-converted; rel err ~4e-3 vs the 2e-2
gate), so the PE does ONLY the two 1024^3 matmul chains (54.6 us floor):

  - H and P transposes run on the DMA XBAR (dma_start(transpose=True),
    bf16-only) instead of the PE.
  - s_dep fold: T1T'[k,i] = T1T[k,i] + w2[k]  (per-partition scalar add on
    the PSUM->SBUF copy) makes mm2 emit sum_k w2[k]*PT[k,j] = s_dep[j].
  - s_head fold: PT'[k,j] = PT[k,j] + v[k] with v = U^-1 w1 (host solve)
    makes mm2 emit sum_k T1T[k,i]*v[k] = head_i @ (U v) = s_head[i].
  - cross term w2.v is a constant, folded with b0 into the epilogue's
    per-partition bias column.

Engines: PE = 256 matmuls only; DVE = T1T' copies + epilogue bias-adds;
GpSimd = U-column SWDGE loads + PT'+v adds; SP + ACT = the two HWDGE DMA
queues (transposing loads, output stores).
"""

import numpy as np
import ml_dtypes

import concourse.bass as bass
import concourse.mybir as mybir
import concourse.tile as tile
from concourse import bacc
from concourse.bass_utils import run_bass_kernel_spmd

B, S, D = 8, 1024, 1024
P = 128
DO = 8       # 1024 / 128
NH = 512     # matmul free-dim tile (one fp32 PSUM bank)
F32 = mybir.dt.float32
BF16 = mybir.dt.bfloat16
ADD = mybir.AluOpType.add
BF = ml_dtypes.bfloat16

_CACHE = {}


def build_nc():
    nc = bacc.Bacc(None, target_bir_lowering=False)

    head = nc.dram_tensor("head", [S, D], BF16, kind="ExternalInput")
    dep = nc.dram_tensor("dep", [S, D], BF16, kind="ExternalInput")
    # u_prep[kt, dd, do, kk] = U[do*128+dd, kt*128+kk]
    edge_u = nc.dram_tensor("edge_u", [DO, P, DO, P], BF16, kind="ExternalInput")
    w2col = nc.dram_tensor("w2col", [P, DO], F32, kind="ExternalInput")
    vcol = nc.dram_tensor("vcol", [P, DO], F32, kind="ExternalInput")
    biascol = nc.dram_tensor("biascol", [P, 1], F32, kind="ExternalInput")
    out = nc.dram_tensor("out", [S, S], F32, kind="ExternalOutput")

    with tile.TileContext(nc) as tc:
        with (
            tc.tile_pool(name="const", bufs=1) as const,
            tc.tile_pool(name="big", bufs=1) as big,
            tc.tile_pool(name="outp", bufs=4) as outp,
            tc.tile_pool(name="mm_ps", bufs=5, space="PSUM") as mm_ps,
        ):
            w2c = const.tile([P, DO], F32)
            vc = const.tile([P, DO], F32)
            bc = const.tile([P, 1], F32)

            u_sb = big.tile([P, DO, DO, P], BF16, tag="u")    # [dd, kt, do, kk]
            ht_sb = big.tile([P, DO, S], BF16, tag="ht")      # [dd, do, i]
            pt_sb = big.tile([P, DO, S], BF16, tag="pt")      # [kk, kt, j]
            ptv_sb = big.tile([P, DO, S], BF16, tag="ptv")    # PT + v
            t1t_sb = big.tile([P, DO, S], BF16, tag="t1t")    # [kk, kt, i]

            # ---------- DMA dispatch (per-queue FIFO; order = priority) -----
            # The XBAR serializes transposes across queues, so give all four
            # HT quarters the XBAR before the PT halves. SP gets the later
            # out stores; ACT carries the PT transposes.
            for q in range(4):
                eng = nc.sync if q % 2 == 0 else nc.scalar
                eng.dma_start(
                    ht_sb[:, :, q * 256:(q + 1) * 256],
                    head[q * 256:(q + 1) * 256, :],
                    transpose=True,
                )
            for jh in range(2):
                nc.scalar.dma_start(
                    pt_sb[:, :, jh * NH:(jh + 1) * NH],
                    dep[jh * NH:(jh + 1) * NH, :],
                    transpose=True,
                )
            # GpSimd SWDGE queue: U column-chunks + the small columns.
            nc.gpsimd.dma_start(u_sb[:, 0], edge_u[0])
            nc.gpsimd.dma_start(u_sb[:, 1], edge_u[1])
            nc.gpsimd.dma_start(w2c[:], w2col[:])
            nc.gpsimd.dma_start(vc[:], vcol[:])
            for kt in range(2, DO):
                nc.gpsimd.dma_start(u_sb[:, kt], edge_u[kt])
            nc.gpsimd.dma_start(bc[:], biascol[:])

            # ---------- helpers ---------------------------------------------
            def ptv_ops(jh):
                # PT' = PT + v on DVE (gpsimd tensor_scalar is ~7.5us/op)
                for kt in range(DO):
                    sl = slice(jh * NH, (jh + 1) * NH)
                    nc.vector.tensor_scalar(
                        ptv_sb[:, kt, sl], pt_sb[:, kt, sl],
                        vc[:, kt:kt + 1], None, ADD,
                    )

            def mm1_group(kt, ih):
                ps = mm_ps.tile([P, NH], F32, tag="mm")
                for do in range(DO):
                    nc.tensor.matmul(
                        ps[:],
                        u_sb[:, kt, do, :],
                        ht_sb[:, do, ih * NH:(ih + 1) * NH],
                        start=(do == 0),
                        stop=(do == DO - 1),
                    )
                nc.vector.tensor_scalar(
                    t1t_sb[:, kt, ih * NH:(ih + 1) * NH],
                    ps[:], w2c[:, kt:kt + 1], None, ADD,
                )

            # ---------- mm1: T1T[k,i] = sum_d U[d,k] HT[d,i]; +w2 on copy ---
            # DVE FIFO order: T1T ih0 copies, PT'+v jh0, PT'+v jh1, T1T ih1
            # copies — keeps every DVE op ready when reached, and PT' jh0
            # complete (~29us) before mm2 starts (~31us).
            for kt in range(DO):
                mm1_group(kt, 0)
            ptv_ops(0)
            ptv_ops(1)
            for kt in range(DO):
                mm1_group(kt, 1)

            # ---------- mm2 + epilogue (+bias col) + store ------------------
            for it in range(DO):
                for jh in range(2):
                    ps = mm_ps.tile([P, NH], F32, tag="mm")
                    for kt in range(DO):
                        nc.tensor.matmul(
                            ps[:],
                            t1t_sb[:, kt, it * P:(it + 1) * P],
                            ptv_sb[:, kt, jh * NH:(jh + 1) * NH],
                            start=(kt == 0),
                            stop=(kt == DO - 1),
                        )
                    ot = outp.tile([P, NH], F32, tag="out")
                    last = (it == DO - 1 and jh == 1)
                    split = 4 if last else 1
                    w = NH // split
                    for s in range(split):
                        sl = slice(s * w, (s + 1) * w)
                        nc.vector.tensor_scalar(
                            ot[:, sl], ps[:, sl], bc[:, 0:1], None, ADD,
                        )
                        eng = nc.scalar if (last and s % 2 == 1) else nc.sync
                        eng.dma_start(
                            out[it * P:(it + 1) * P,
                                jh * NH + s * w:jh * NH + (s + 1) * w],
                            ot[:, sl],
                        )

    nc.compile()
    return nc


def _get_nc():
    if "nc" not in _CACHE:
        _CACHE["nc"] = build_nc()
    return _CACHE["nc"]


def _in_maps(head, dep, edge_U, edge_W, edge_b):
    head = np.asarray(head, dtype=np.float32)
    dep = np.asarray(dep, dtype=np.float32)
    U = np.asarray(edge_U, dtype=np.float32)
    w = np.asarray(edge_W, dtype=np.float32).reshape(-1)
    w1, w2 = w[:D], w[D:]
    b0 = float(np.asarray(edge_b, dtype=np.float32).reshape(-1)[0])

    Ub = U.astype(BF)
    # v = U^-1 w1 against the bf16-rounded U the device actually uses, so
    # sum_k T1T[k,i] v[k] reproduces head_i @ w1 exactly up to bf16 noise.
    v = np.linalg.solve(Ub.astype(np.float64), w1.astype(np.float64))

    u_prep = np.ascontiguousarray(
        Ub.reshape(DO, P, DO, P).transpose(2, 1, 0, 3)
    )
    w2col = np.ascontiguousarray(w2.reshape(DO, P).T)
    vcol = np.ascontiguousarray(v.astype(np.float32).reshape(DO, P).T)
    biascol = np.full((P, 1), b0 - float(w2.astype(np.float64) @ v),
                      dtype=np.float32)

    maps = []
    for b in range(B):
        maps.append({
            "head": np.ascontiguousarray(head[b]).astype(BF),
            "dep": np.ascontiguousarray(dep[b]).astype(BF),
            "edge_u": u_prep,
            "w2col": w2col,
            "vcol": vcol,
            "biascol": biascol,
        })
    return maps


def kernel(head, dep, edge_U, edge_W, edge_b, **run_kwargs):
    nc = _get_nc()
    maps = _in_maps(head, dep, edge_U, edge_W, edge_b)
    res = run_bass_kernel_spmd(nc, maps, core_ids=list(range(B)), **run_kwargs)
    out = np.stack([np.asarray(res.results[c]["out"]) for c in range(B)], axis=0)
    if run_kwargs:
        _CACHE["last_result"] = res
    return out
